# revision 2
# baseline (speedup 1.0000x reference)
"""Trainium2 Bass kernel for nn_AdaptiveFullConnected (segment_reduce).

Reference computation (per batch b):
    c      = coords + depthwise_conv1d(coords, K=5) + conv_b          [N, 2]
    h      = gelu(c @ lin1_w.T + lin1_b)                              [N, 512]
    weight = h @ lin2_w.T + lin2_b                                    [N, 512]
    xw     = tile(x, 8) * weight                                      [N, 512]
    mean_p = mean over {n : idx[n] == p} of xw[n, :]                  [P, 512]
    out    = w1 * sin(mean) + w2 * cos(mean)                          [P, 512]

Sharding: 8 cores = (batch b = core//2) x (half of N = core%2), 8192 rows
per core.  lin2 and the segment one-hot matmuls run fp8e4 DoubleRow; the
depthwise conv, the one-hot matrix (host-built, DMA'd in), the lin2-bias
fold (b2 * seg(x) / count) and 1/count are computed on the host.

No cross-core communication: each core computes sin/cos of its PARTIAL
segment mean (its 8192 rows' contribution, scaled by the full-batch
1/count, plus half the bias term) for all 256 segments, and the host
combines the two halves of each pair during unsharding with the angle
addition identities:
    sin(mA+mB) = sinA cosB + cosA sinB
    cos(mA+mB) = cosA cosB - sinA sinB

The lin1 bias is folded into the matmul as a third contraction row
(ones row in ct), so gelu needs no per-block bias and the one-hot comes
from HBM instead of a DVE is_equal pass.  ct is DMA'd in 8 chunks (it
lives on only 3 partitions, ~2.6 GB/s per partition line) so the main
loop starts ~2.5us in instead of waiting ~9us for one serial transfer.

The main loop is software-pipelined: lin1+gelu one chunk ahead, segment
matmuls one pair behind, so the in-order engines never stall on the
cross-engine PE -> ACT -> PE -> DVE -> PE chain.
"""

import numpy as np
from contextlib import ExitStack

B = 4
N = 16384
DIMS = 64
HEADS = 8
D = DIMS * HEADS  # 512
K = 5
PFULL = 256
NCORES = 8
NLOC = N // 2  # 8192 rows per core
NT = NLOC // 128  # 64 n-tiles
CHUNK = 512
NCH = NLOC // CHUNK  # 16
NPAIR = NT // 2  # 32

_CACHE = {}


def build_nc():
    import concourse.bass as bass  # noqa: F401
    import concourse.mybir as mybir
    import concourse.tile as tile
    from concourse import bacc

    f16 = mybir.dt.float16
    f32 = mybir.dt.float32
    f8 = mybir.dt.float8e4
    DR = mybir.MatmulPerfMode.DoubleRow
    mult = mybir.AluOpType.mult
    add = mybir.AluOpType.add
    AF = mybir.ActivationFunctionType

    nc = bacc.Bacc("TRN2", num_devices=NCORES)

    ct16 = nc.declare_dram_parameter("ct16", [3, NLOC], f16, isOutput=False)
    w1a = nc.declare_dram_parameter("w1a", [3, D], f16, isOutput=False)
    w2dr = nc.declare_dram_parameter("w2dr", [128, 4 * D], f8, isOutput=False)
    x16 = nc.declare_dram_parameter("x16", [128, NT * DIMS], f16, isOutput=False)
    oh8 = nc.declare_dram_parameter(
        "oh8", [128, NPAIR * 2 * PFULL], f8, isOutput=False
    )
    bm16 = nc.declare_dram_parameter("bm16", [128, 2 * D], f16, isOutput=False)
    consts = nc.declare_dram_parameter("consts", [128, 16], f32, isOutput=False)
    out = nc.declare_dram_parameter("out", [128, 4 * D], f16, isOutput=True)

    with tile.TileContext(nc, num_cores=NCORES) as tc, ExitStack() as ctx:
        cpool = ctx.enter_context(tc.tile_pool(name="cpool", bufs=1))
        work = ctx.enter_context(tc.tile_pool(name="work", bufs=1))
        psum = ctx.enter_context(tc.tile_pool(name="psum", bufs=1, space="PSUM"))

        # ---- critical-path loads on the sync ring: consts, lin1 weights,
        # then ct in 8 chunks so lin1 chunk 0 can start ~2.5us in ----
        cst = cpool.tile([128, 16], f32)
        nc.sync.dma_start(out=cst[:], in_=consts[:])
        w1_sb = cpool.tile([3, D], f16)
        nc.sync.dma_start(out=w1_sb[:], in_=w1a[:])
        ct_sb = cpool.tile([3, NLOC], f16)
        CTC = NLOC // 8
        for q in range(8):
            nc.sync.dma_start(
                out=ct_sb[:, q * CTC : (q + 1) * CTC],
                in_=ct16[:, q * CTC : (q + 1) * CTC],
            )

        # ---- bulk loads on the gpsimd ring (engine otherwise idle; the
        # scalar ring would steal ACT time) in consumption order ----
        w2_sb = cpool.tile([128, 2, 2, D], f8)
        nc.gpsimd.dma_start(
            out=w2_sb[:], in_=w2dr[:].rearrange("p (g j d) -> p g j d", j=2, d=D)
        )
        x_sb = cpool.tile([128, NT, DIMS], f16)
        xv3 = x16[:].rearrange("p (t c) -> p t c", c=DIMS)
        nc.gpsimd.dma_start(out=x_sb[:, 0:8], in_=xv3[:, 0:8])
        oh_sb = cpool.tile([128, NPAIR, 2, PFULL], f8)
        ohv = oh8[:].rearrange("p (q j s) -> p q j s", j=2, s=PFULL)
        nc.gpsimd.dma_start(out=oh_sb[:, 0:8], in_=ohv[:, 0:8])
        nc.gpsimd.dma_start(out=x_sb[:, 8:32], in_=xv3[:, 8:32])
        nc.gpsimd.dma_start(out=oh_sb[:, 8:16], in_=ohv[:, 8:16])
        bm_sb = cpool.tile([128, 2, D], f16)
        nc.gpsimd.dma_start(
            out=bm_sb[:], in_=bm16[:].rearrange("p (h d) -> p h d", d=D)
        )
        # remaining bulk on the sync ring (after the ct chunks)
        nc.sync.dma_start(out=x_sb[:, 32:64], in_=xv3[:, 32:64])
        nc.sync.dma_start(out=oh_sb[:, 16:32], in_=ohv[:, 16:32])

        # PE warm-up: the HAM activity gate throttles a cold PE; burn junk
        # matmuls so the loop is entered warm with no PE idle gap
        zt = cpool.tile([128, 256], f16)
        nc.gpsimd.memset(zt[:], 0.0)
        pwarm = psum.tile([128, 256], f32, name="pwarm", tag="ph", bufs=2)
        for _ in range(8):
            nc.tensor.matmul(
                pwarm[:], lhsT=zt[:, 0:128], rhs=zt[:], start=True, stop=True
            )

        # persistent segment accumulators for segments [0:128] and [128:256]
        pseg = [psum.tile([128, D], f32, name=f"pseg{i}") for i in range(2)]

        # ---- main loop, software-pipelined ----
        # The per-tile chain PE(lin1) -> ACT(gelu) -> PE(lin2) -> DVE(mult)
        # -> PE(seg) serializes if emitted in dataflow order (in-order
        # engines: the seg matmul blocks PE's stream on the DVE multiply).
        # So lin1+gelu are emitted one chunk ahead and the seg matmuls one
        # pair behind, giving every cross-engine hop a full pair-time of
        # slack.
        def alloc_htg():
            return [
                work.tile([128, 2, CHUNK], f8, name=f"htg{g}", bufs=2)
                for g in range(2)
            ]

        def emit_lin1(c, htg, es):
            for e in es:
                g, j = e // 2, e % 2
                ph = psum.tile([128, CHUNK], f32, name="ph", bufs=2)
                nc.tensor.matmul(
                    ph[:],
                    lhsT=w1_sb[:, e * 128 : (e + 1) * 128],
                    rhs=ct_sb[:, c * CHUNK : (c + 1) * CHUNK],
                    start=True, stop=True,
                )
                nc.scalar.activation(
                    out=htg[g][:, j, :], in_=ph[:], func=AF.Gelu
                )

        def emit_seg(p):
            kt0, xwp, pair = p
            for p2 in range(2):
                nc.tensor.matmul(
                    pseg[p2][:],
                    lhsT=oh_sb[:, pair, :, p2 * 128 : (p2 + 1) * 128],
                    rhs=xwp[:],
                    start=(pair == 0), stop=(pair == NPAIR - 1),
                    perf_mode=DR,
                )

        htg_cur = alloc_htg()
        emit_lin1(0, htg_cur, range(4))
        pending = None  # (kt0, xwp, pair) awaiting its seg matmuls
        for c in range(NCH):
            htg = htg_cur
            htg_next = alloc_htg() if c + 1 < NCH else None
            # the pending seg matmuls go between the two lin1 halves: their
            # ~0.5us of PE work covers exactly the time gelu needs to drain
            # the first ph buffers before lin1 e2/e3 reuse them
            if htg_next is not None:
                emit_lin1(c + 1, htg_next, (0, 1))
                if pending is not None:
                    emit_seg(pending)
                    pending = None
                emit_lin1(c + 1, htg_next, (2, 3))
            for tp in range(2):
                pair = 2 * c + tp
                kt0 = 4 * c + 2 * tp
                xwp = work.tile([128, 2, D], f8, name="xwp", bufs=3)
                pw = psum.tile([128, 2, D], f32, name="pw", bufs=2)
                for h2 in range(2):
                    t4 = 2 * tp + h2
                    for g in range(2):
                        nc.tensor.matmul(
                            pw[:, h2, :],
                            lhsT=htg[g][:, :, t4 * 128 : (t4 + 1) * 128],
                            rhs=w2_sb[:, g],
                            start=(g == 0), stop=(g == 1), perf_mode=DR,
                        )
                xv = (
                    x_sb[:, kt0 : kt0 + 2, :]
                    .unsqueeze(2)
                    .to_broadcast([128, 2, HEADS, DIMS])
                )
                nc.vector.tensor_tensor(
                    out=xwp[:].rearrange("p h (hh c) -> p h hh c", c=DIMS),
                    in0=pw[:].rearrange("p h (hh c) -> p h hh c", c=DIMS),
                    in1=xv, op=mult,
                )
                if pending is not None:
                    emit_seg(pending)
                pending = (kt0, xwp, pair)
            htg_cur = htg_next
        emit_seg(pending)

        # ---- epilogue: partial means -> sin/cos halves, host combines.
        # Each half's output DMA is issued as soon as that half is done so
        # it overlaps the other half's trig.
        out_sb = work.tile([128, 4, D], f16, name="out_sb")
        ov4 = out[:].rearrange("p (h d) -> p h d", d=D)
        for p2 in range(2):
            mean = work.tile([128, D], f32, name=f"mean{p2}")
            nc.vector.scalar_tensor_tensor(
                out=mean[:], in0=pseg[p2][:], scalar=cst[:, p2 : p2 + 1],
                in1=bm_sb[:, p2, :], op0=mult, op1=add,
            )
            nc.scalar.activation(
                out=out_sb[:, p2, :], in_=mean[:], func=AF.Sin
            )
            nc.scalar.activation(
                out=out_sb[:, 2 + p2, :], in_=mean[:], func=AF.Sin,
                bias=cst[:, 2:3],
            )
            nc.sync.dma_start(out=ov4[:, p2], in_=out_sb[:, p2, :])
            nc.scalar.dma_start(out=ov4[:, 2 + p2], in_=out_sb[:, 2 + p2, :])

    nc.finalize()
    return nc


def make_in_maps(x, coords, indices, conv_w, conv_b, lin1_w, lin1_b, lin2_w,
                 lin2_b, w1, w2):
    """Host-side sharding + layout prep.  Returns list of 8 input dicts."""
    import ml_dtypes

    f8 = ml_dtypes.float8_e4m3

    x = np.asarray(x, np.float32)
    coords = np.asarray(coords, np.float32)
    idx_full = np.asarray(indices).reshape(B, N).astype(np.int32)
    conv_w = np.asarray(conv_w, np.float32)
    conv_b = np.asarray(conv_b, np.float32)
    lin1_w = np.asarray(lin1_w, np.float32)
    lin1_b = np.asarray(lin1_b, np.float32)
    lin2_w = np.asarray(lin2_w, np.float32)
    lin2_b = np.asarray(lin2_b, np.float32)
    _CACHE["w1s"] = np.float32(np.asarray(w1).reshape(-1)[0])
    _CACHE["w2s"] = np.float32(np.asarray(w2).reshape(-1)[0])

    # depthwise conv on host (layout prep for the c-channel input)
    cpad = np.zeros((B, N + 4, 2), np.float32)
    cpad[:, 2:-2] = coords
    c2 = coords + conv_b[None, None, :]
    for k in range(K):
        c2 = c2 + cpad[:, k : k + N, :] * conv_w[:, 0, k][None, None, :]

    # full-batch segment stats for the bias fold
    xseg = np.zeros((B, PFULL, DIMS), np.float32)
    counts = np.zeros((B, PFULL), np.float32)
    for b in range(B):
        np.add.at(xseg[b], idx_full[b], x[b])
        np.add.at(counts[b], idx_full[b], 1.0)

    # lin1 weights + bias as a K=3 contraction (ones row appended to ct)
    w1a = np.concatenate(
        [lin1_w.T, lin1_b[None, :]], axis=0
    ).astype(np.float16)  # [3, D]
    w2t = np.ascontiguousarray(lin2_w.T)  # [f, d]
    w2dr = (
        w2t.reshape(2, 2, 128, D).transpose(2, 0, 1, 3).reshape(128, 4 * D)
        .astype(f8)
    )

    seg_ids = np.arange(PFULL, dtype=np.int32)
    in_maps = []
    for core in range(NCORES):
        b, half = core // 2, core % 2
        lo = half * NLOC
        ct3 = np.empty((3, NLOC), np.float32)
        ct3[0:2] = c2[b, lo : lo + NLOC, :].T
        ct3[2] = 1.0
        ct16 = ct3.astype(np.float16)
        xs = x[b, lo : lo + NLOC, :]
        x_sh = (
            xs.reshape(NT, 128, DIMS).transpose(1, 0, 2).reshape(128, NT * DIMS)
            .astype(np.float16)
        )
        idx_sh = np.ascontiguousarray(
            idx_full[b, lo : lo + NLOC].reshape(NT, 128).T
        ).astype(np.int32)  # [128, NT]
        # host-built one-hot: oh[r, pair, j, s] = (idx[r, 2*pair+j] == s)
        oh = (
            idx_sh.reshape(128, NPAIR, 2, 1) == seg_ids[None, None, None, :]
        ).astype(f8).reshape(128, NPAIR * 2 * PFULL)
        cnt = counts[b]  # [256]
        # each half adds HALF the bias-fold term; the two halves' partial
        # means sum to the full mean on the host
        bmf = (
            0.5 * lin2_b[None, :] * np.tile(xseg[b], (1, HEADS)) / cnt[:, None]
        )  # [256, 512]
        bm = np.ascontiguousarray(
            bmf.reshape(2, 128, D).transpose(1, 0, 2).reshape(128, 2 * D)
        ).astype(np.float16)
        consts = np.zeros((128, 16), np.float32)
        consts[:, 0] = 1.0 / cnt[0:128]
        consts[:, 1] = 1.0 / cnt[128:256]
        consts[:, 2] = np.pi / 2
        in_maps.append(
            dict(
                ct16=ct16, w1a=w1a, w2dr=w2dr, x16=x_sh, oh8=oh, bm16=bm,
                consts=consts,
            )
        )
    return in_maps


def assemble(results):
    """[8 x {'out': [128, 2048]}] -> [B, PFULL, D] float32 via the angle
    addition identities (each core produced sin/cos of its partial mean)."""
    w1s, w2s = _CACHE["w1s"], _CACHE["w2s"]
    out = np.empty((B, PFULL, D), np.float32)
    for b in range(B):
        ra = results[2 * b]["out"].astype(np.float32).reshape(128, 4, D)
        rb = results[2 * b + 1]["out"].astype(np.float32).reshape(128, 4, D)
        sa = np.concatenate([ra[:, 0, :], ra[:, 1, :]], axis=0)  # [256, D]
        ca = np.concatenate([ra[:, 2, :], ra[:, 3, :]], axis=0)
        sb = np.concatenate([rb[:, 0, :], rb[:, 1, :]], axis=0)
        cb = np.concatenate([rb[:, 2, :], rb[:, 3, :]], axis=0)
        sin_t = sa * cb + ca * sb
        cos_t = ca * cb - sa * sb
        out[b] = w1s * sin_t + w2s * cos_t
    return out


def kernel(x, coords, indices, patch_seq_len, conv_w, conv_b, lin1_w, lin1_b,
           lin2_w, lin2_b, w1, w2):
    from concourse.bass_utils import run_bass_kernel_spmd

    if "nc" not in _CACHE:
        _CACHE["nc"] = build_nc()
    nc = _CACHE["nc"]
    in_maps = make_in_maps(x, coords, indices, conv_w, conv_b, lin1_w, lin1_b,
                           lin2_w, lin2_b, w1, w2)
    res = run_bass_kernel_spmd(nc, in_maps, core_ids=list(range(NCORES)))
    return assemble(res.results)


# revision 3
# speedup vs baseline: 1.1744x; 1.1744x over previous
"""Trainium2 Bass kernel for nn_AdaptiveFullConnected (segment_reduce).

Reference computation (per batch b):
    c      = coords + depthwise_conv1d(coords, K=5) + conv_b          [N, 2]
    h      = gelu(c @ lin1_w.T + lin1_b)                              [N, 512]
    weight = h @ lin2_w.T + lin2_b                                    [N, 512]
    xw     = tile(x, 8) * weight                                      [N, 512]
    mean_p = mean over {n : idx[n] == p} of xw[n, :]                  [P, 512]
    out    = w1 * sin(mean) + w2 * cos(mean)                          [P, 512]

Sharding: 8 cores = (batch b = core//2) x (half of N = core%2), 8192 rows
per core.  lin2 and the segment one-hot matmuls run fp8e4 DoubleRow; the
depthwise conv, the one-hot matrix (host-built, DMA'd in), the lin2-bias
fold (b2 * seg(x) / count) and 1/count are computed on the host.

No cross-core communication: each core computes sin/cos of its PARTIAL
segment mean (its 8192 rows' contribution, scaled by the full-batch
1/count, plus half the bias term) for all 256 segments, and the host
combines the two halves of each pair during unsharding with the angle
addition identities:
    sin(mA+mB) = sinA cosB + cosA sinB
    cos(mA+mB) = cosA cosB - sinA sinB

The lin1 bias is folded into the matmul as a third contraction row
(ones row in ct), so gelu needs no per-block bias and the one-hot comes
from HBM instead of a DVE is_equal pass.  ct is DMA'd in 8 chunks (it
lives on only 3 partitions, ~2.6 GB/s per partition line) so the main
loop starts ~2.5us in instead of waiting ~9us for one serial transfer.

The main loop is software-pipelined: lin1+gelu one chunk ahead, segment
matmuls one pair behind, so the in-order engines never stall on the
cross-engine PE -> ACT -> PE -> DVE -> PE chain.
"""

import numpy as np
from contextlib import ExitStack

B = 4
N = 16384
DIMS = 64
HEADS = 8
D = DIMS * HEADS  # 512
K = 5
PFULL = 256
NCORES = 8
NLOC = N // 2  # 8192 rows per core
NT = NLOC // 128  # 64 n-tiles
CHUNK = 512
NCH = NLOC // CHUNK  # 16
NPAIR = NT // 2  # 32

_CACHE = {}


def build_nc():
    import concourse.bass as bass  # noqa: F401
    import concourse.mybir as mybir
    import concourse.tile as tile
    from concourse import bacc

    f16 = mybir.dt.float16
    f32 = mybir.dt.float32
    f8 = mybir.dt.float8e4
    DR = mybir.MatmulPerfMode.DoubleRow
    mult = mybir.AluOpType.mult
    add = mybir.AluOpType.add
    AF = mybir.ActivationFunctionType

    nc = bacc.Bacc("TRN2", num_devices=NCORES)

    ct16 = nc.declare_dram_parameter("ct16", [3, NLOC], f16, isOutput=False)
    w1a = nc.declare_dram_parameter("w1a", [3, D], f16, isOutput=False)
    w2dr = nc.declare_dram_parameter("w2dr", [128, 4 * D], f8, isOutput=False)
    x16 = nc.declare_dram_parameter("x16", [128, NT * DIMS], f16, isOutput=False)
    oh8 = nc.declare_dram_parameter(
        "oh8", [128, NPAIR * 2 * PFULL], f8, isOutput=False
    )
    bm16 = nc.declare_dram_parameter("bm16", [128, 2 * D], f16, isOutput=False)
    consts = nc.declare_dram_parameter("consts", [128, 16], f32, isOutput=False)
    out = nc.declare_dram_parameter("out", [128, 4 * D], f16, isOutput=True)

    with tile.TileContext(nc, num_cores=NCORES) as tc, ExitStack() as ctx:
        cpool = ctx.enter_context(tc.tile_pool(name="cpool", bufs=1))
        work = ctx.enter_context(tc.tile_pool(name="work", bufs=1))
        psum = ctx.enter_context(tc.tile_pool(name="psum", bufs=1, space="PSUM"))

        # ---- critical-path loads on the sync ring: consts, lin1 weights,
        # then ct in 8 chunks so lin1 chunk 0 can start ~2.5us in ----
        cst = cpool.tile([128, 16], f32)
        nc.sync.dma_start(out=cst[:], in_=consts[:])
        w1_sb = cpool.tile([3, D], f16)
        nc.sync.dma_start(out=w1_sb[:], in_=w1a[:])
        ct_sb = cpool.tile([3, NLOC], f16)
        CTC = NLOC // 8
        for q in range(8):
            nc.sync.dma_start(
                out=ct_sb[:, q * CTC : (q + 1) * CTC],
                in_=ct16[:, q * CTC : (q + 1) * CTC],
            )

        # ---- bulk loads on the gpsimd ring (engine otherwise idle; the
        # scalar ring would steal ACT time) in consumption order ----
        w2_sb = cpool.tile([128, 2, 2, D], f8)
        nc.gpsimd.dma_start(
            out=w2_sb[:], in_=w2dr[:].rearrange("p (g j d) -> p g j d", j=2, d=D)
        )
        x_sb = cpool.tile([128, NT, DIMS], f16)
        xv3 = x16[:].rearrange("p (t c) -> p t c", c=DIMS)
        nc.gpsimd.dma_start(out=x_sb[:, 0:8], in_=xv3[:, 0:8])
        oh_sb = cpool.tile([128, NPAIR, 2, PFULL], f8)
        ohv = oh8[:].rearrange("p (q j s) -> p q j s", j=2, s=PFULL)
        nc.gpsimd.dma_start(out=oh_sb[:, 0:8], in_=ohv[:, 0:8])
        nc.gpsimd.dma_start(out=x_sb[:, 8:32], in_=xv3[:, 8:32])
        nc.gpsimd.dma_start(out=oh_sb[:, 8:16], in_=ohv[:, 8:16])
        bm_sb = cpool.tile([128, 2, D], f16)
        nc.gpsimd.dma_start(
            out=bm_sb[:], in_=bm16[:].rearrange("p (h d) -> p h d", d=D)
        )
        # remaining bulk on the sync ring (after the ct chunks)
        nc.sync.dma_start(out=x_sb[:, 32:64], in_=xv3[:, 32:64])
        nc.sync.dma_start(out=oh_sb[:, 16:32], in_=ohv[:, 16:32])

        # PE warm-up: the HAM clock gate holds the PE at 1.2 GHz until one
        # FULLY-busy free-running 4096-cycle window (~3.4us) is observed.
        # The main loop has cross-engine stalls, so at cold speed it can go
        # 30-120us without ever presenting a gap-free window.  Burn ~7.5us
        # of back-to-back junk matmuls (>= 2 full windows regardless of
        # phase) so the 8/8 grant fires HERE, then enter the loop with no
        # PE idle gap; the loop's own activity holds the grant.
        zt = cpool.tile([128, 512], f16)
        nc.gpsimd.memset(zt[:], 0.0)
        pwarm = psum.tile([128, 512], f32, name="pwarm", tag="ph", bufs=2)
        for _ in range(16):
            nc.tensor.matmul(
                pwarm[:], lhsT=zt[:, 0:128], rhs=zt[:], start=True, stop=True
            )

        # persistent segment accumulators for segments [0:128] and [128:256]
        pseg = [psum.tile([128, D], f32, name=f"pseg{i}") for i in range(2)]

        # ---- main loop, software-pipelined ----
        # The per-tile chain PE(lin1) -> ACT(gelu) -> PE(lin2) -> DVE(mult)
        # -> PE(seg) serializes if emitted in dataflow order (in-order
        # engines: the seg matmul blocks PE's stream on the DVE multiply).
        # So lin1+gelu are emitted one chunk ahead and the seg matmuls one
        # pair behind, giving every cross-engine hop a full pair-time of
        # slack.
        def alloc_htg():
            return [
                work.tile([128, 2, CHUNK], f8, name=f"htg{g}", bufs=2)
                for g in range(2)
            ]

        def emit_lin1(c, htg, es):
            for e in es:
                g, j = e // 2, e % 2
                ph = psum.tile([128, CHUNK], f32, name="ph", bufs=2)
                nc.tensor.matmul(
                    ph[:],
                    lhsT=w1_sb[:, e * 128 : (e + 1) * 128],
                    rhs=ct_sb[:, c * CHUNK : (c + 1) * CHUNK],
                    start=True, stop=True,
                )
                nc.scalar.activation(
                    out=htg[g][:, j, :], in_=ph[:], func=AF.Gelu
                )

        def emit_seg(p):
            kt0, xwp, pair = p
            for p2 in range(2):
                nc.tensor.matmul(
                    pseg[p2][:],
                    lhsT=oh_sb[:, pair, :, p2 * 128 : (p2 + 1) * 128],
                    rhs=xwp[:],
                    start=(pair == 0), stop=(pair == NPAIR - 1),
                    perf_mode=DR,
                )

        htg_cur = alloc_htg()
        emit_lin1(0, htg_cur, range(4))
        pending = None  # (kt0, xwp, pair) awaiting its seg matmuls
        for c in range(NCH):
            htg = htg_cur
            htg_next = alloc_htg() if c + 1 < NCH else None
            # the pending seg matmuls go between the two lin1 halves: their
            # ~0.5us of PE work covers exactly the time gelu needs to drain
            # the first ph buffers before lin1 e2/e3 reuse them
            if htg_next is not None:
                emit_lin1(c + 1, htg_next, (0, 1))
                if pending is not None:
                    emit_seg(pending)
                    pending = None
                emit_lin1(c + 1, htg_next, (2, 3))
            for tp in range(2):
                pair = 2 * c + tp
                kt0 = 4 * c + 2 * tp
                xwp = work.tile([128, 2, D], f8, name="xwp", bufs=3)
                pw = psum.tile([128, 2, D], f32, name="pw", bufs=2)
                for h2 in range(2):
                    t4 = 2 * tp + h2
                    for g in range(2):
                        nc.tensor.matmul(
                            pw[:, h2, :],
                            lhsT=htg[g][:, :, t4 * 128 : (t4 + 1) * 128],
                            rhs=w2_sb[:, g],
                            start=(g == 0), stop=(g == 1), perf_mode=DR,
                        )
                xv = (
                    x_sb[:, kt0 : kt0 + 2, :]
                    .unsqueeze(2)
                    .to_broadcast([128, 2, HEADS, DIMS])
                )
                nc.vector.tensor_tensor(
                    out=xwp[:].rearrange("p h (hh c) -> p h hh c", c=DIMS),
                    in0=pw[:].rearrange("p h (hh c) -> p h hh c", c=DIMS),
                    in1=xv, op=mult,
                )
                if pending is not None:
                    emit_seg(pending)
                pending = (kt0, xwp, pair)
            htg_cur = htg_next
        emit_seg(pending)

        # ---- epilogue: partial means -> sin/cos halves, host combines.
        # Each half's output DMA is issued as soon as that half is done so
        # it overlaps the other half's trig.
        out_sb = work.tile([128, 4, D], f16, name="out_sb")
        ov4 = out[:].rearrange("p (h d) -> p h d", d=D)
        for p2 in range(2):
            mean = work.tile([128, D], f32, name=f"mean{p2}")
            nc.vector.scalar_tensor_tensor(
                out=mean[:], in0=pseg[p2][:], scalar=cst[:, p2 : p2 + 1],
                in1=bm_sb[:, p2, :], op0=mult, op1=add,
            )
            nc.scalar.activation(
                out=out_sb[:, p2, :], in_=mean[:], func=AF.Sin
            )
            nc.scalar.activation(
                out=out_sb[:, 2 + p2, :], in_=mean[:], func=AF.Sin,
                bias=cst[:, 2:3],
            )
            nc.sync.dma_start(out=ov4[:, p2], in_=out_sb[:, p2, :])
            nc.scalar.dma_start(out=ov4[:, 2 + p2], in_=out_sb[:, 2 + p2, :])

    nc.finalize()
    return nc


def make_in_maps(x, coords, indices, conv_w, conv_b, lin1_w, lin1_b, lin2_w,
                 lin2_b, w1, w2):
    """Host-side sharding + layout prep.  Returns list of 8 input dicts."""
    import ml_dtypes

    f8 = ml_dtypes.float8_e4m3

    x = np.asarray(x, np.float32)
    coords = np.asarray(coords, np.float32)
    idx_full = np.asarray(indices).reshape(B, N).astype(np.int32)
    conv_w = np.asarray(conv_w, np.float32)
    conv_b = np.asarray(conv_b, np.float32)
    lin1_w = np.asarray(lin1_w, np.float32)
    lin1_b = np.asarray(lin1_b, np.float32)
    lin2_w = np.asarray(lin2_w, np.float32)
    lin2_b = np.asarray(lin2_b, np.float32)
    _CACHE["w1s"] = np.float32(np.asarray(w1).reshape(-1)[0])
    _CACHE["w2s"] = np.float32(np.asarray(w2).reshape(-1)[0])

    # depthwise conv on host (layout prep for the c-channel input)
    cpad = np.zeros((B, N + 4, 2), np.float32)
    cpad[:, 2:-2] = coords
    c2 = coords + conv_b[None, None, :]
    for k in range(K):
        c2 = c2 + cpad[:, k : k + N, :] * conv_w[:, 0, k][None, None, :]

    # full-batch segment stats for the bias fold
    xseg = np.zeros((B, PFULL, DIMS), np.float32)
    counts = np.zeros((B, PFULL), np.float32)
    for b in range(B):
        np.add.at(xseg[b], idx_full[b], x[b])
        np.add.at(counts[b], idx_full[b], 1.0)

    # lin1 weights + bias as a K=3 contraction (ones row appended to ct)
    w1a = np.concatenate(
        [lin1_w.T, lin1_b[None, :]], axis=0
    ).astype(np.float16)  # [3, D]
    w2t = np.ascontiguousarray(lin2_w.T)  # [f, d]
    w2dr = (
        w2t.reshape(2, 2, 128, D).transpose(2, 0, 1, 3).reshape(128, 4 * D)
        .astype(f8)
    )

    seg_ids = np.arange(PFULL, dtype=np.int32)
    in_maps = []
    for core in range(NCORES):
        b, half = core // 2, core % 2
        lo = half * NLOC
        ct3 = np.empty((3, NLOC), np.float32)
        ct3[0:2] = c2[b, lo : lo + NLOC, :].T
        ct3[2] = 1.0
        ct16 = ct3.astype(np.float16)
        xs = x[b, lo : lo + NLOC, :]
        x_sh = (
            xs.reshape(NT, 128, DIMS).transpose(1, 0, 2).reshape(128, NT * DIMS)
            .astype(np.float16)
        )
        idx_sh = np.ascontiguousarray(
            idx_full[b, lo : lo + NLOC].reshape(NT, 128).T
        ).astype(np.int32)  # [128, NT]
        # host-built one-hot: oh[r, pair, j, s] = (idx[r, 2*pair+j] == s)
        oh = (
            idx_sh.reshape(128, NPAIR, 2, 1) == seg_ids[None, None, None, :]
        ).astype(f8).reshape(128, NPAIR * 2 * PFULL)
        cnt = counts[b]  # [256]
        # each half adds HALF the bias-fold term; the two halves' partial
        # means sum to the full mean on the host
        bmf = (
            0.5 * lin2_b[None, :] * np.tile(xseg[b], (1, HEADS)) / cnt[:, None]
        )  # [256, 512]
        bm = np.ascontiguousarray(
            bmf.reshape(2, 128, D).transpose(1, 0, 2).reshape(128, 2 * D)
        ).astype(np.float16)
        consts = np.zeros((128, 16), np.float32)
        consts[:, 0] = 1.0 / cnt[0:128]
        consts[:, 1] = 1.0 / cnt[128:256]
        consts[:, 2] = np.pi / 2
        in_maps.append(
            dict(
                ct16=ct16, w1a=w1a, w2dr=w2dr, x16=x_sh, oh8=oh, bm16=bm,
                consts=consts,
            )
        )
    return in_maps


def assemble(results):
    """[8 x {'out': [128, 2048]}] -> [B, PFULL, D] float32 via the angle
    addition identities (each core produced sin/cos of its partial mean)."""
    w1s, w2s = _CACHE["w1s"], _CACHE["w2s"]
    out = np.empty((B, PFULL, D), np.float32)
    for b in range(B):
        ra = results[2 * b]["out"].astype(np.float32).reshape(128, 4, D)
        rb = results[2 * b + 1]["out"].astype(np.float32).reshape(128, 4, D)
        sa = np.concatenate([ra[:, 0, :], ra[:, 1, :]], axis=0)  # [256, D]
        ca = np.concatenate([ra[:, 2, :], ra[:, 3, :]], axis=0)
        sb = np.concatenate([rb[:, 0, :], rb[:, 1, :]], axis=0)
        cb = np.concatenate([rb[:, 2, :], rb[:, 3, :]], axis=0)
        sin_t = sa * cb + ca * sb
        cos_t = ca * cb - sa * sb
        out[b] = w1s * sin_t + w2s * cos_t
    return out


def kernel(x, coords, indices, patch_seq_len, conv_w, conv_b, lin1_w, lin1_b,
           lin2_w, lin2_b, w1, w2):
    from concourse.bass_utils import run_bass_kernel_spmd

    if "nc" not in _CACHE:
        _CACHE["nc"] = build_nc()
    nc = _CACHE["nc"]
    in_maps = make_in_maps(x, coords, indices, conv_w, conv_b, lin1_w, lin1_b,
                           lin2_w, lin2_b, w1, w2)
    res = run_bass_kernel_spmd(nc, in_maps, core_ids=list(range(NCORES)))
    return assemble(res.results)
